# revision 13
# baseline (speedup 1.0000x reference)
"""Edge-parallel GNN message-passing kernel for 8 trn2 NeuronCores.

Computation (see reference):
    p = x @ Wp + bp   [N,1]
    c = x @ Wc + bc   [N,1]
    out[e] = |p[dst[e]] - c[src[e]]| * W1 + b1   for each edge e

Strategy (v3):
  - Edges are assigned to the core that OWNS their dst node, so the p-side
    values are local.  Nodes are dealt round-robin by dst-degree so cores
    get ~12500 nodes and ~75000 edges each.
  - Host orders each core's nodes into per-partition dst-degree buckets:
    bucket v holds (128 x m_v) nodes of degree v, so the edge grid columns
    [C_v, C_v + m_v*v) hold each node's v edges contiguously and the
    p-value expansion is a free stride-0 broadcast view of the projection
    output (zero gather descriptors for the dst side).
  - x arrives host-pre-transposed [128ch x slots]; phase 1 is a plain
    matmul per 128-node tile (no PE transposes / PSUM copies).
  - Only the c-projection is AllGathered (f32, one value per node slot).
    It is then spread into 256B-strided rows of 8 values and the src side
    is fetched with 32B dma_gather elements (descriptor-rate bound either
    way, so small elements keep HBM traffic low) + 1-of-8 mask select.
  - Fused sub/abs/affine tail; host inverts the slot layout.
"""

import numpy as np

import concourse.bacc as bacc
import concourse.tile as tile
from concourse import bass, mybir
from concourse import bass_utils
from gather32 import raw_dma_gather

N_CORES = 8
N_NODES = 100000
N_EDGES = 600000
IN_CH = 128

NI_CHUNK = 8192

F32 = mybir.dt.float32
I16 = mybir.dt.int16

_CACHED = {}
_LAST_RES = None
_LAST_PLAN = None


def _build_nc(J_ALL, S2, buckets):
    """buckets: list of (v, m_v, j0, c0) with v>=1."""
    NPC2 = J_ALL * 128
    G2 = N_CORES * NPC2
    T8_ROWS = G2 // 8           # multiple of 128 since J_ALL*8 cores
    RPP = T8_ROWS // 128        # rows per partition in the spread tile
    IDX_COLS = (128 * S2) // 16

    nc = bacc.Bacc("TRN2", target_bir_lowering=False, debug=False,
                   num_devices=N_CORES, num_swdge_queues=4)

    xt = nc.dram_tensor("xt", [IN_CH, NPC2], F32, kind="ExternalInput")
    qs = nc.dram_tensor("qs", [128, IDX_COLS], I16, kind="ExternalInput")
    rs = nc.dram_tensor("rs", [128, S2], F32, kind="ExternalInput")
    w = nc.dram_tensor("w", [IN_CH, 2], F32, kind="ExternalInput")
    scal = nc.dram_tensor("scal", [128, 16], F32, kind="ExternalInput")
    out = nc.dram_tensor("out", [128 * S2], F32, kind="ExternalOutput")

    with tile.TileContext(nc) as tc:
        with (
            tc.tile_pool(name="cst", bufs=1) as cst,
            tc.tile_pool(name="sb", bufs=3) as sb,
            tc.tile_pool(name="edge", bufs=1) as edge,
            tc.tile_pool(name="ps", bufs=2, space="PSUM") as ps,
            tc.tile_pool(name="pcps", bufs=1, space="PSUM") as pcps,
            tc.tile_pool(name="gat", bufs=3) as gat,
            tc.tile_pool(name="dram", bufs=1, space="DRAM") as dram,
        ):
            from concourse.masks import make_identity
            ident = cst.tile([128, 128], F32)
            make_identity(nc, ident[:])
            w_sb = cst.tile([IN_CH, 2], F32)
            nc.sync.dma_start(out=w_sb[:], in_=w[:])
            scal_sb = cst.tile([128, 16], F32)
            nc.sync.dma_start(out=scal_sb[:], in_=scal[:])

            qs_sb = edge.tile([128, IDX_COLS], I16)
            nc.sync.dma_start(out=qs_sb[:], in_=qs[:])
            rs_sb = edge.tile([128, S2], F32)
            nc.sync.dma_start(out=rs_sb[:], in_=rs[:])

            # ---- phase 1: project nodes; xt already [ch, slot] on host
            CH = 8                       # tiles per load chunk
            n_chunks = (J_ALL + CH - 1) // CH
            xt_r = xt.rearrange("c (j n) -> c j n", n=128)
            pc_ps = pcps.tile([128, 2 * J_ALL], F32)
            for g in range(n_chunks):
                j0 = g * CH
                j1 = min(J_ALL, j0 + CH)
                xc = sb.tile([128, CH, 128], F32, tag="xc")
                nc.sync.dma_start(out=xc[:, : j1 - j0, :], in_=xt_r[:, j0:j1])
                for j in range(j0, j1):
                    nc.tensor.matmul(
                        out=pc_ps[:, 2 * j:2 * j + 2],
                        lhsT=xc[:, j - j0, :],
                        rhs=w_sb[:],
                        start=True,
                        stop=True,
                    )
            pc_sb = cst.tile([128, 2 * J_ALL], F32)
            nc.vector.tensor_copy(pc_sb[:], pc_ps[:])

            # ---- phase 2: c column -> node-slot-ordered row, bounce to DRAM
            bounce = dram.tile([NPC2], F32)
            for t0 in range(0, J_ALL, 128):
                t1 = min(J_ALL, t0 + 128)
                cp_ps = ps.tile([128, 128], F32, tag="cp")
                nc.tensor.transpose(
                    cp_ps[: t1 - t0, :],
                    pc_sb[:, 2 * t0 + 1:2 * t1:2],
                    ident[:],
                )
                row = sb.tile([128, 128], F32, tag="row")
                nc.vector.tensor_copy(row[: t1 - t0, :], cp_ps[: t1 - t0, :])
                nc.sync.dma_start(
                    out=bounce[t0 * 128:t1 * 128].rearrange(
                        "(t p) -> t p", p=128),
                    in_=row[: t1 - t0, :],
                )

            # ---- phase 3: all-gather the c table (one f32 per node slot)
            ctab = dram.tile([N_CORES, NPC2], F32)
            nc.gpsimd.collective_compute(
                "AllGather",
                mybir.AluOpType.bypass,
                replica_groups=[list(range(N_CORES))],
                ins=[bounce.opt()],
                outs=[ctab.opt()],
            )

            # ---- phase 3b: spread into 256B-strided rows of 8 values
            t8c = dram.tile([T8_ROWS, 64], F32)
            g2_sb = cst.tile([128, RPP * 8], F32)
            nc.sync.dma_start(
                out=g2_sb[:],
                in_=ctab.rearrange("a n -> (a n)")
                .rearrange("(p f) -> p f", p=128),
            )
            t8c_sb = cst.tile([128, RPP * 64], F32)
            nc.gpsimd.memset(t8c_sb[:], 0.0)
            nc.vector.tensor_copy(
                out=t8c_sb[:].rearrange("p (r e) -> p r e", e=64)[:, :, 0:8],
                in_=g2_sb[:].rearrange("p (r e) -> p r e", e=8),
            )
            nc.sync.dma_start(
                out=t8c.rearrange("(p r) e -> p (r e)", p=128),
                in_=t8c_sb[:],
            )

            # ---- phase 4: src gather (32B rows) + 1-of-8 select
            val_s = edge.tile([128, S2], F32)
            iota_b = scal_sb[:, 0:8]
            EPC_PAD = 128 * S2
            n_full = EPC_PAD // NI_CHUNK
            widths = [NI_CHUNK // 128] * n_full
            rem = EPC_PAD - n_full * NI_CHUNK
            if rem:
                widths.append(rem // 128)
            i0 = 0
            gather_no = 0
            for wdt in widths:
                ni = wdt * 128
                icol0 = i0 * 8
                gth = gat.tile([128, NI_CHUNK // 128, 8], F32, tag="gth")
                raw_dma_gather(
                    nc.gpsimd,
                    out_ap=gth[:, :wdt, :],
                    in_ap=t8c[:, 0:8],
                    idxs_ap=qs_sb[:, icol0:icol0 + wdt * 8],
                    num_idxs=ni,
                    elem_size=8,
                    elem_step=64,
                    queue_num=gather_no % 4,
                )
                gather_no += 1
                msk = gat.tile([128, NI_CHUNK // 128, 8], F32, tag="msk")
                nc.vector.tensor_tensor(
                    out=msk[:, :wdt, :],
                    in0=iota_b.rearrange("p (one e) -> p one e", one=1)
                    .broadcast_to([128, wdt, 8]),
                    in1=rs_sb[:, i0:i0 + wdt]
                    .rearrange("p (i one) -> p i one", one=1)
                    .broadcast_to([128, wdt, 8]),
                    op=mybir.AluOpType.is_equal,
                )
                nc.vector.tensor_tensor(
                    out=msk[:, :wdt, :],
                    in0=msk[:, :wdt, :],
                    in1=gth[:, :wdt, :],
                    op=mybir.AluOpType.mult,
                )
                nc.vector.tensor_reduce(
                    out=val_s[:, i0:i0 + wdt],
                    in_=msk[:, :wdt, :],
                    axis=mybir.AxisListType.X,
                    op=mybir.AluOpType.add,
                )
                i0 += wdt

            # ---- phase 5: per-bucket broadcast subtract, then abs/affine
            sub = edge.tile([128, S2], F32)
            res = edge.tile([128, S2], F32)
            for v, m_v, j0, c0 in buckets:
                nc.vector.tensor_tensor(
                    out=sub[:, c0:c0 + m_v * v]
                    .rearrange("p (m k) -> p m k", k=v),
                    in0=pc_sb[:, 2 * j0:2 * (j0 + m_v):2]
                    .rearrange("p (m one) -> p m one", one=1)
                    .broadcast_to([128, m_v, v]),
                    in1=val_s[:, c0:c0 + m_v * v]
                    .rearrange("p (m k) -> p m k", k=v),
                    op=mybir.AluOpType.subtract,
                )
            nc.scalar.activation(
                out=sub[:], in_=sub[:],
                func=mybir.ActivationFunctionType.Abs,
                bias=scal_sb[:, 8:9], scale=1.0,
            )
            nc.vector.scalar_tensor_tensor(
                out=res[:], in0=sub[:],
                scalar=scal_sb[:, 9:10],
                in1=scal_sb[:, 10:11].to_broadcast([128, S2]),
                op0=mybir.AluOpType.mult,
                op1=mybir.AluOpType.add,
            )
            nc.sync.dma_start(
                out=out.rearrange("(p s) -> p s", s=S2), in_=res[:]
            )

    nc.compile()
    return nc


def _wrap16(stream):
    w = stream.reshape(-1, 16).T
    return np.tile(w, (8, 1))


def _plan(adjs):
    """Host-side layout planning from the edge list alone."""
    src = np.asarray(adjs[0], dtype=np.int64)
    dst = np.asarray(adjs[1], dtype=np.int64)
    cnt = np.bincount(dst, minlength=N_NODES)

    # deal nodes round-robin by descending degree -> balanced nodes & edges
    node_order = np.argsort(-cnt, kind="stable")
    core_of_node = np.empty(N_NODES, dtype=np.int64)
    core_of_node[node_order] = np.arange(N_NODES) % N_CORES

    vmax = int(cnt.max())
    # bucket capacities m_v (shared across cores): max per-core node count
    n_v = np.zeros((N_CORES, vmax + 1), dtype=np.int64)
    for k in range(N_CORES):
        nodes_k = node_order[k::N_CORES]
        n_v[k] = np.bincount(cnt[nodes_k], minlength=vmax + 1)
    m_v = (n_v.max(axis=0) + 127) // 128          # nodes per partition

    # column layout: buckets v = vmax..1 get edge columns, v=0 only slots
    buckets = []          # (v, m_v, j0, c0)
    j0 = 0
    c0 = 0
    j0_of = {}
    c0_of = {}
    for v in range(vmax, 0, -1):
        if m_v[v] == 0:
            continue
        buckets.append((v, int(m_v[v]), j0, c0))
        j0_of[v] = j0
        c0_of[v] = c0
        j0 += int(m_v[v])
        c0 += int(m_v[v]) * v
    S2 = c0
    if m_v[0]:
        j0_of[0] = j0
        j0 += int(m_v[0])
    J_ALL = j0
    NPC2 = J_ALL * 128

    j0_lut = np.zeros(vmax + 1, dtype=np.int64)
    c0_lut = np.zeros(vmax + 1, dtype=np.int64)
    for v, j in j0_of.items():
        j0_lut[v] = j
    for v, c in c0_of.items():
        c0_lut[v] = c

    # per-node slot assignment (p, j) and edge slot assignment (p, col)
    node_p = np.empty(N_NODES, dtype=np.int64)
    node_j = np.empty(N_NODES, dtype=np.int64)
    for k in range(N_CORES):
        nodes_k = node_order[k::N_CORES]
        cv = cnt[nodes_k]
        # stable sort by descending count groups nodes by bucket
        o = np.argsort(-cv, kind="stable")
        nk = nodes_k[o]
        cvk = cv[o]
        # rank within bucket
        start = np.searchsorted(-cvk, -cvk, side="left")
        rank = np.arange(len(nk)) - start
        node_p[nk] = rank % 128
        node_j[nk] = j0_lut[cvk] + rank // 128

    edge_core = core_of_node[dst]
    # rank of each edge within its dst node
    ds = np.argsort(dst, kind="stable")
    starts = np.searchsorted(dst[ds], dst[ds], side="left")
    t_of = np.empty(N_EDGES, dtype=np.int64)
    t_of[ds] = np.arange(N_EDGES) - starts

    dv = cnt[dst]
    c0_arr = c0_lut[dv]
    j0_arr = j0_lut[dv]
    edge_p = node_p[dst]
    edge_col = c0_arr + (node_j[dst] - j0_arr) * dv + t_of

    # src table position of each edge
    cpos = core_of_node[src] * NPC2 + node_j[src] * 128 + node_p[src]

    return dict(
        cnt=cnt, node_order=node_order, core_of_node=core_of_node,
        node_p=node_p, node_j=node_j, edge_core=edge_core, edge_p=edge_p,
        edge_col=edge_col, cpos=cpos, buckets=buckets, S2=S2, J_ALL=J_ALL,
        NPC2=NPC2,
    )


def kernel(x, adjs, Wp, bp, Wc, bc, W1, b1):
    global _LAST_RES, _LAST_PLAN
    x = np.ascontiguousarray(np.asarray(x, dtype=np.float32))
    adjs = np.asarray(adjs)
    Wp = np.asarray(Wp, dtype=np.float32)
    bp = np.asarray(bp, dtype=np.float32)
    Wc = np.asarray(Wc, dtype=np.float32)
    bc = np.asarray(bc, dtype=np.float32)
    W1 = np.asarray(W1, dtype=np.float32)
    b1 = np.asarray(b1, dtype=np.float32)

    plan = _plan(adjs)
    _LAST_PLAN = plan
    S2, J_ALL, NPC2 = plan["S2"], plan["J_ALL"], plan["NPC2"]

    w = np.concatenate([Wp, Wc], axis=1)
    scal = np.zeros((128, 16), dtype=np.float32)
    scal[:, 0:8] = np.arange(8, dtype=np.float32)[None, :]
    scal[:, 8] = bp[0] - bc[0]
    scal[:, 9] = W1[0, 0]
    scal[:, 10] = b1[0]

    in_maps = []
    for k in range(N_CORES):
        mask_n = plan["core_of_node"] == k
        nodes = np.where(mask_n)[0]
        slot = plan["node_j"][nodes] * 128 + plan["node_p"][nodes]
        xt = np.zeros((NPC2, IN_CH), dtype=np.float32)
        xt[slot] = x[nodes]
        xt = np.ascontiguousarray(xt.T)          # [128ch, NPC2]

        mask_e = plan["edge_core"] == k
        ep = plan["edge_p"][mask_e]
        ec = plan["edge_col"][mask_e]
        cp = plan["cpos"][mask_e]
        qrow = np.zeros((128, S2), dtype=np.int64)
        lane = np.zeros((128, S2), dtype=np.float32)
        qrow[ep, ec] = cp >> 3
        lane[ep, ec] = (cp & 7).astype(np.float32)
        stream = qrow.T.reshape(-1)              # j = i*128 + p
        in_maps.append({
            "xt": xt,
            "qs": _wrap16(stream.astype(np.int16)),
            "rs": lane,
            "w": w,
            "scal": scal,
        })

    key = (J_ALL, S2, tuple(plan["buckets"]))
    if key not in _CACHED:
        _CACHED.clear()
        _CACHED[key] = _build_nc(J_ALL, S2, plan["buckets"])
    nc = _CACHED[key]

    res = bass_utils.run_bass_kernel_spmd(
        nc, in_maps, core_ids=list(range(N_CORES))
    )
    _LAST_RES = res

    out = np.empty(N_EDGES, dtype=np.float32)
    for k in range(N_CORES):
        mask_e = plan["edge_core"] == k
        o2d = res.results[k]["out"].reshape(128, S2)
        out[mask_e] = o2d[plan["edge_p"][mask_e], plan["edge_col"][mask_e]]
    return out


# revision 22
# speedup vs baseline: 2.0815x; 2.0815x over previous
"""Edge-parallel GNN message-passing kernel for 8 trn2 NeuronCores.

Computation (see reference):
    p = x @ Wp + bp   [N,1]
    c = x @ Wc + bc   [N,1]
    out[e] = |p[dst[e]] - c[src[e]]| * W1 + b1   for each edge e

Strategy (v3):
  - Edges are assigned to the core that OWNS their dst node, so the p-side
    values are local.  Nodes are dealt round-robin by dst-degree so cores
    get ~12500 nodes and ~75000 edges each.
  - Host orders each core's nodes into per-partition dst-degree buckets:
    bucket v holds (128 x m_v) nodes of degree v, so the edge grid columns
    [C_v, C_v + m_v*v) hold each node's v edges contiguously and the
    p-value expansion is a free stride-0 broadcast view of the projection
    output (zero gather descriptors for the dst side).
  - x arrives host-pre-transposed [128ch x slots]; phase 1 is a plain
    matmul per 128-node tile (no PE transposes / PSUM copies).
  - Only the c-projection is AllGathered (f32, one value per node slot).
    It is then spread into 256B-strided rows of 8 values and the src side
    is fetched with 32B dma_gather elements (descriptor-rate bound either
    way, so small elements keep HBM traffic low) + 1-of-8 mask select.
  - Fused sub/abs/affine tail; host inverts the slot layout.
"""

import numpy as np

import concourse.bacc as bacc
import concourse.tile as tile
from concourse import bass, mybir
from concourse import bass_utils
from gather32 import raw_dma_gather

N_CORES = 8
N_NODES = 100000
N_EDGES = 600000
IN_CH = 128

NI_CHUNK = 8192

F32 = mybir.dt.float32
I16 = mybir.dt.int16

_CACHED = {}
_LAST_RES = None
_LAST_PLAN = None


def _build_nc(J_ALL, S2, buckets, gather_reps=1, skip_ag=False,
              skip_t8c=False, do_gather=True, do_select=True):
    """buckets: list of (v, m_v, j0, c0) with v>=1."""
    NPC2 = J_ALL * 128
    G2 = N_CORES * NPC2
    T8_ROWS = G2 // 8           # multiple of 128 since J_ALL*8 cores
    RPP = T8_ROWS // 128        # rows per partition in the spread tile
    IDX_COLS = (128 * S2) // 16

    nc = bacc.Bacc("TRN2", target_bir_lowering=False, debug=False,
                   num_devices=N_CORES, num_swdge_queues=4)

    xt = nc.dram_tensor("xt", [IN_CH, NPC2], F32, kind="ExternalInput")
    qs = nc.dram_tensor("qs", [128, IDX_COLS], I16, kind="ExternalInput")
    rs = nc.dram_tensor("rs", [128, S2], F32, kind="ExternalInput")
    w = nc.dram_tensor("w", [IN_CH, 2], F32, kind="ExternalInput")
    scal = nc.dram_tensor("scal", [128, 16], F32, kind="ExternalInput")
    out = nc.dram_tensor("out", [128 * S2], F32, kind="ExternalOutput")

    with tile.TileContext(nc) as tc:
        with (
            tc.tile_pool(name="cst", bufs=1) as cst,
            tc.tile_pool(name="sb", bufs=3) as sb,
            tc.tile_pool(name="edge", bufs=1) as edge,
            tc.tile_pool(name="ps", bufs=2, space="PSUM") as ps,
            tc.tile_pool(name="pcps", bufs=1, space="PSUM") as pcps,
            tc.tile_pool(name="gat", bufs=3) as gat,
            tc.tile_pool(name="dram", bufs=1, space="DRAM") as dram,
        ):
            from concourse.masks import make_identity
            ident = cst.tile([128, 128], F32)
            make_identity(nc, ident[:])
            w_sb = cst.tile([IN_CH, 2], F32)
            nc.sync.dma_start(out=w_sb[:], in_=w[:])
            scal_sb = cst.tile([128, 16], F32)
            nc.sync.dma_start(out=scal_sb[:], in_=scal[:])

            qs_sb = edge.tile([128, IDX_COLS], I16)
            nc.sync.dma_start(out=qs_sb[:], in_=qs[:])
            rs_sb = edge.tile([128, S2], F32)
            nc.sync.dma_start(out=rs_sb[:], in_=rs[:])

            # ---- phase 1: project nodes; xt already [ch, slot] on host
            CH = 8                       # tiles per load chunk
            n_chunks = (J_ALL + CH - 1) // CH
            xt_r = xt.rearrange("c (j n) -> c j n", n=128)
            pc_ps = pcps.tile([128, 2 * J_ALL], F32)
            for g in range(n_chunks):
                j0 = g * CH
                j1 = min(J_ALL, j0 + CH)
                xc = sb.tile([128, CH, 128], F32, tag="xc")
                nc.sync.dma_start(out=xc[:, : j1 - j0, :], in_=xt_r[:, j0:j1])
                for j in range(j0, j1):
                    nc.tensor.matmul(
                        out=pc_ps[:, 2 * j:2 * j + 2],
                        lhsT=xc[:, j - j0, :],
                        rhs=w_sb[:],
                        start=True,
                        stop=True,
                    )
            pc_sb = cst.tile([128, 2 * J_ALL], F32)
            nc.vector.tensor_copy(pc_sb[:], pc_ps[:])

            # ---- phase 2: c column -> node-slot-ordered row, bounce to DRAM
            bounce = dram.tile([NPC2], F32)
            for t0 in range(0, J_ALL, 128):
                t1 = min(J_ALL, t0 + 128)
                cp_ps = ps.tile([128, 128], F32, tag="cp")
                nc.tensor.transpose(
                    cp_ps[: t1 - t0, :],
                    pc_sb[:, 2 * t0 + 1:2 * t1:2],
                    ident[:],
                )
                row = sb.tile([128, 128], F32, tag="row")
                nc.vector.tensor_copy(row[: t1 - t0, :], cp_ps[: t1 - t0, :])
                nc.sync.dma_start(
                    out=bounce[t0 * 128:t1 * 128].rearrange(
                        "(t p) -> t p", p=128),
                    in_=row[: t1 - t0, :],
                )

            # ---- phase 3: all-gather the c table (one f32 per node slot)
            ctab = dram.tile([N_CORES, NPC2], F32)
            if not skip_ag:
                nc.gpsimd.collective_compute(
                    "AllGather",
                    mybir.AluOpType.bypass,
                    replica_groups=[list(range(N_CORES))],
                    ins=[bounce.opt()],
                    outs=[ctab.opt()],
                )

            # ---- phase 3b: spread into 256B-strided rows of 8 values
            t8c = dram.tile([T8_ROWS, 64], F32)
            if not skip_t8c:
                g2_sb = cst.tile([128, RPP * 8], F32)
                nc.sync.dma_start(
                    out=g2_sb[:],
                    in_=ctab.rearrange("a n -> (a n)")
                    .rearrange("(p f) -> p f", p=128),
                )
                t8c_sb = cst.tile([128, RPP * 64], F32)
                nc.gpsimd.memset(t8c_sb[:], 0.0)
                nc.vector.tensor_copy(
                    out=t8c_sb[:].rearrange("p (r e) -> p r e", e=64)[:, :, 0:8],
                    in_=g2_sb[:].rearrange("p (r e) -> p r e", e=8),
                )
                nc.sync.dma_start(
                    out=t8c.rearrange("(p r) e -> p (r e)", p=128),
                    in_=t8c_sb[:],
                )

            # ---- phase 4: src gather (32B rows) + 1-of-8 select
            val_s = edge.tile([128, S2], F32)
            if gather_reps == 0:
                nc.vector.memset(val_s[:], 0.0)
            iota_b = scal_sb[:, 0:8]
            EPC_PAD = 128 * S2
            n_full = EPC_PAD // NI_CHUNK
            widths = [NI_CHUNK // 128] * n_full
            rem = EPC_PAD - n_full * NI_CHUNK
            if rem:
                widths.append(rem // 128)
            gather_no = 0
            for _rep in range(gather_reps):
              i0 = 0
              for wdt in widths:
                ni = wdt * 128
                icol0 = i0 * 8
                gth = gat.tile([128, NI_CHUNK // 128, 8], F32, tag="gth")
                if do_gather:
                    raw_dma_gather(
                        nc.gpsimd,
                        out_ap=gth[:, :wdt, :],
                        in_ap=t8c[:, 0:8],
                        idxs_ap=qs_sb[:, icol0:icol0 + wdt * 8],
                        num_idxs=ni,
                        elem_size=8,
                        elem_step=64,
                        queue_num=gather_no % 4,
                    )
                else:
                    nc.vector.memset(gth[:, :wdt, :], 0.0)
                gather_no += 1
                if do_select:
                    msk = gat.tile([128, NI_CHUNK // 128, 8], F32, tag="msk")
                    nc.vector.tensor_tensor(
                        out=msk[:, :wdt, :],
                        in0=iota_b.rearrange("p (one e) -> p one e", one=1)
                        .broadcast_to([128, wdt, 8]),
                        in1=rs_sb[:, i0:i0 + wdt]
                        .rearrange("p (i one) -> p i one", one=1)
                        .broadcast_to([128, wdt, 8]),
                        op=mybir.AluOpType.is_equal,
                    )
                    nc.vector.tensor_tensor(
                        out=msk[:, :wdt, :],
                        in0=msk[:, :wdt, :],
                        in1=gth[:, :wdt, :],
                        op=mybir.AluOpType.mult,
                    )
                    nc.vector.tensor_reduce(
                        out=val_s[:, i0:i0 + wdt],
                        in_=msk[:, :wdt, :],
                        axis=mybir.AxisListType.X,
                        op=mybir.AluOpType.add,
                    )
                else:
                    nc.vector.tensor_reduce(
                        out=val_s[:, i0:i0 + wdt],
                        in_=gth[:, :wdt, :],
                        axis=mybir.AxisListType.X,
                        op=mybir.AluOpType.add,
                    )
                i0 += wdt

            # ---- phase 5: per-bucket broadcast subtract, then abs/affine
            sub = edge.tile([128, S2], F32)
            res = edge.tile([128, S2], F32)
            for v, m_v, j0, c0 in buckets:
                nc.vector.tensor_tensor(
                    out=sub[:, c0:c0 + m_v * v]
                    .rearrange("p (m k) -> p m k", k=v),
                    in0=pc_sb[:, 2 * j0:2 * (j0 + m_v):2]
                    .rearrange("p (m one) -> p m one", one=1)
                    .broadcast_to([128, m_v, v]),
                    in1=val_s[:, c0:c0 + m_v * v]
                    .rearrange("p (m k) -> p m k", k=v),
                    op=mybir.AluOpType.subtract,
                )
            nc.scalar.activation(
                out=sub[:], in_=sub[:],
                func=mybir.ActivationFunctionType.Abs,
                bias=scal_sb[:, 8:9], scale=1.0,
            )
            nc.vector.scalar_tensor_tensor(
                out=res[:], in0=sub[:],
                scalar=scal_sb[:, 9:10],
                in1=scal_sb[:, 10:11].to_broadcast([128, S2]),
                op0=mybir.AluOpType.mult,
                op1=mybir.AluOpType.add,
            )
            nc.sync.dma_start(
                out=out.rearrange("(p s) -> p s", s=S2), in_=res[:]
            )

    nc.compile()
    return nc


def _wrap16(stream):
    w = stream.reshape(-1, 16).T
    return np.tile(w, (8, 1))


V_CAP = 8     # max bucket degree; higher-degree nodes split into virtuals


def _plan(adjs):
    """Host-side layout planning from the edge list alone."""
    src = np.asarray(adjs[0], dtype=np.int64)
    dst = np.asarray(adjs[1], dtype=np.int64)
    cnt = np.bincount(dst, minlength=N_NODES)

    # deal nodes round-robin by descending degree -> balanced nodes & edges
    node_order = np.argsort(-cnt, kind="stable")
    core_of_node = np.empty(N_NODES, dtype=np.int64)
    core_of_node[node_order] = np.arange(N_NODES) % N_CORES

    # split nodes into virtual nodes of degree <= V_CAP
    nv = np.maximum((cnt + V_CAP - 1) // V_CAP, 1)
    first = np.zeros(N_NODES, dtype=np.int64)
    np.cumsum(nv[:-1], out=first[1:])
    NV = int(nv.sum())
    virt_owner = np.repeat(np.arange(N_NODES), nv)
    virt_rank = np.arange(NV) - first[virt_owner]
    virt_cnt = np.minimum(cnt[virt_owner] - virt_rank * V_CAP, V_CAP)
    virt_core = core_of_node[virt_owner]

    # bucket capacities m_v (shared across cores): max per-core virt count
    n_v = np.zeros((N_CORES, V_CAP + 1), dtype=np.int64)
    for k in range(N_CORES):
        n_v[k] = np.bincount(virt_cnt[virt_core == k], minlength=V_CAP + 1)
    m_v = (n_v.max(axis=0) + 127) // 128          # virtuals per partition

    # column layout: buckets v = V_CAP..1 get edge columns, v=0 only slots
    buckets = []          # (v, m_v, j0, c0)
    j0 = 0
    c0 = 0
    j0_lut = np.zeros(V_CAP + 1, dtype=np.int64)
    c0_lut = np.zeros(V_CAP + 1, dtype=np.int64)
    for v in range(V_CAP, 0, -1):
        if m_v[v] == 0:
            continue
        buckets.append((v, int(m_v[v]), j0, c0))
        j0_lut[v] = j0
        c0_lut[v] = c0
        j0 += int(m_v[v])
        c0 += int(m_v[v]) * v
    S2 = c0
    if m_v[0]:
        j0_lut[0] = j0
        j0 += int(m_v[0])
    J_ALL = j0
    NPC2 = J_ALL * 128

    # per-virtual slot assignment (p, j)
    virt_p = np.empty(NV, dtype=np.int64)
    virt_j = np.empty(NV, dtype=np.int64)
    for k in range(N_CORES):
        vids = np.where(virt_core == k)[0]
        cv = virt_cnt[vids]
        o = np.argsort(-cv, kind="stable")
        vk = vids[o]
        cvk = cv[o]
        start = np.searchsorted(-cvk, -cvk, side="left")
        rank = np.arange(len(vk)) - start
        virt_p[vk] = rank % 128
        virt_j[vk] = j0_lut[cvk] + rank // 128

    edge_core = core_of_node[dst]
    # rank of each edge within its dst node
    ds = np.argsort(dst, kind="stable")
    starts = np.searchsorted(dst[ds], dst[ds], side="left")
    t_of = np.empty(N_EDGES, dtype=np.int64)
    t_of[ds] = np.arange(N_EDGES) - starts

    evirt = first[dst] + t_of // V_CAP
    tprime = t_of % V_CAP
    ev = virt_cnt[evirt]
    edge_p = virt_p[evirt]
    edge_col = c0_lut[ev] + (virt_j[evirt] - j0_lut[ev]) * ev + tprime

    # src table position of each edge (first virtual of the src node)
    sv = first[src]
    cpos = core_of_node[src] * NPC2 + virt_j[sv] * 128 + virt_p[sv]

    # per-virtual x slot (for building xt) and c-table slot of each virtual
    return dict(
        cnt=cnt, node_order=node_order, core_of_node=core_of_node,
        virt_owner=virt_owner, virt_core=virt_core, virt_p=virt_p,
        virt_j=virt_j, edge_core=edge_core, edge_p=edge_p,
        edge_col=edge_col, cpos=cpos, buckets=buckets, S2=S2, J_ALL=J_ALL,
        NPC2=NPC2,
    )


def kernel(x, adjs, Wp, bp, Wc, bc, W1, b1):
    global _LAST_RES, _LAST_PLAN
    x = np.ascontiguousarray(np.asarray(x, dtype=np.float32))
    adjs = np.asarray(adjs)
    Wp = np.asarray(Wp, dtype=np.float32)
    bp = np.asarray(bp, dtype=np.float32)
    Wc = np.asarray(Wc, dtype=np.float32)
    bc = np.asarray(bc, dtype=np.float32)
    W1 = np.asarray(W1, dtype=np.float32)
    b1 = np.asarray(b1, dtype=np.float32)

    plan = _plan(adjs)
    _LAST_PLAN = plan
    S2, J_ALL, NPC2 = plan["S2"], plan["J_ALL"], plan["NPC2"]

    w = np.concatenate([Wp, Wc], axis=1)
    scal = np.zeros((128, 16), dtype=np.float32)
    scal[:, 0:8] = np.arange(8, dtype=np.float32)[None, :]
    scal[:, 8] = bp[0] - bc[0]
    scal[:, 9] = W1[0, 0]
    scal[:, 10] = b1[0]

    T8_ROWS = NPC2  # 8 * NPC2 table entries / 8 per row
    # pad slots get indices spread across the table (a single hot row
    # serializes the DRAM bank and doubles the gather time)
    spread = (np.arange(128 * S2, dtype=np.int64) * 37) % T8_ROWS

    in_maps = []
    for k in range(N_CORES):
        mask_v = plan["virt_core"] == k
        vids = np.where(mask_v)[0]
        slot = plan["virt_j"][vids] * 128 + plan["virt_p"][vids]
        xt = np.zeros((NPC2, IN_CH), dtype=np.float32)
        xt[slot] = x[plan["virt_owner"][vids]]
        xt = np.ascontiguousarray(xt.T)          # [128ch, NPC2]

        mask_e = plan["edge_core"] == k
        ep = plan["edge_p"][mask_e]
        ec = plan["edge_col"][mask_e]
        cp = plan["cpos"][mask_e]
        qrow = spread.reshape(128, S2).copy()
        lane = np.zeros((128, S2), dtype=np.float32)
        qrow[ep, ec] = cp >> 3
        lane[ep, ec] = (cp & 7).astype(np.float32)
        stream = qrow.T.reshape(-1)              # j = i*128 + p
        in_maps.append({
            "xt": xt,
            "qs": _wrap16(stream.astype(np.int16)),
            "rs": lane,
            "w": w,
            "scal": scal,
        })

    key = (J_ALL, S2, tuple(plan["buckets"]))
    if key not in _CACHED:
        _CACHED.clear()
        _CACHED[key] = _build_nc(J_ALL, S2, plan["buckets"])
    nc = _CACHED[key]

    res = bass_utils.run_bass_kernel_spmd(
        nc, in_maps, core_ids=list(range(N_CORES))
    )
    _LAST_RES = res

    out = np.empty(N_EDGES, dtype=np.float32)
    for k in range(N_CORES):
        mask_e = plan["edge_core"] == k
        o2d = res.results[k]["out"].reshape(128, S2)
        out[mask_e] = o2d[plan["edge_p"][mask_e], plan["edge_col"][mask_e]]
    return out
